# revision 22
# baseline (speedup 1.0000x reference)
"""PointLaplacianLoss kernel for Trainium2 (8 NeuronCores, Bass/Tile).

Problem (hardcoded shapes): point1, point2: (B=4, N=8192, D=3) fp32.
  knn_idx = 8 nearest neighbors of each point1 row (self excluded),
  lap(p) = mean_k p[knn_idx] - p,  out = mean(|lap(p1) - lap(p2)|).

Algebraic simplification: with q = p1 - p2,
  lap(p1) - lap(p2) = mean_k q[knn_idx] - q
so only one gather table (q) is needed.

Sharding: 2 cores per batch; each core handles 4096 rows of one batch's
8192x8192 distance matrix.  All cores run the same program; each core's
host prep rotates the column order by its first row index r0 (local
column jj <-> global (jj + r0) % N), which puts every row's self column
on the compile-time block diagonal.  The gather table q is rotated
identically so device-side local indices address it directly.

Per core:
  - PE computes -d2 tiles via a K=13 float32r matmul.  float32r has a
    reduced mantissa, so coordinates are split hi/lo (hi = fp32 with the
    low 13 mantissa bits zeroed, exactly representable in float32r;
    lo = p - hi).  2<p_i,p_j> = 2(hi_i.hi_j + hi_i.lo_j + lo_i.hi_j)
    (the lo.lo term ~1e-6 is dropped), and the squared norms ride along
    as split constant rows, giving fp32-grade -d2 at ~1 cycle/row
    instead of fp32's 4.
  - ScalarE copies PSUM -> SBUF m_tile (keeps VectorE free for scans)
  - VectorE masks self (block-diag add), then max / max_index give the
    top-8 values + column indices per row
  - neighbors are gathered with one-offset-per-partition indirect DMAs
    (the SWDGE ucode mishandles multi-offset APs), barrier-separated
    from compute (concurrent GPSIMD descriptor generation and DVE SBUF
    traffic crashes the device)
  - VectorE reduces to per-partition L1 partials; PE reduces across
    partitions via a ones matmul; host sums 8 scalars and divides.
"""

import numpy as np

import concourse.bass as bass
import concourse.mybir as mybir
from concourse import bacc
from concourse.bass_utils import run_bass_kernel_spmd
from concourse.tile import TileContext

B, N, D = 4, 8192, 3
K = 8
N_CORES = 8
ROWS_PER_CORE = N * B // N_CORES  # 4096
RB = 128  # rows per block (partition count)
N_RB = ROWS_PER_CORE // RB  # 32
CHUNK = 512  # psum free-dim chunk (one bank of fp32)
N_CHUNK = N // CHUNK  # 16
MM_K = 13  # contraction rows of the hi/lo split matmul
NEG_BIG = -1e30

_CACHED = {}


def build_nc(
    loop_reps: int = 1,
    for_sim: bool = False,
    use_f32r: bool = True,
    n_swdge: int = 4,
):
    nc = bacc.Bacc("TRN2", target_bir_lowering=False, num_swdge_queues=n_swdge)
    f32 = mybir.dt.float32
    mm_dt = mybir.dt.float32r if use_f32r else f32
    u32 = mybir.dt.uint32

    p_mat = nc.declare_dram_parameter(
        "mat", [MM_K, ROWS_PER_CORE + N], mm_dt, isOutput=False
    )
    p_q = nc.declare_dram_parameter("q", [N, D], f32, isOutput=False)
    p_aux = nc.declare_dram_parameter("aux", [RB, N_RB * D + RB], f32, isOutput=False)

    o_partial = nc.declare_dram_parameter("partial", [1, 1], f32, isOutput=True)
    o_idx = nc.declare_dram_parameter("idx", [RB, N_RB * K], u32, isOutput=True)

    with nc.semaphore("gsem") as gsem, TileContext(nc) as tc:
        with (
            tc.tile_pool(name="singles", bufs=1) as singles,
            tc.tile_pool(name="mtiles", bufs=2) as mpool,
            tc.tile_pool(name="psum", bufs=7, space="PSUM") as pp,
            tc.tile_pool(name="psum_out", bufs=1, space="PSUM") as pp_out,
            tc.tile_pool(name="small", bufs=2) as small,
        ):
            mat = singles.tile([MM_K, ROWS_PER_CORE + N], mm_dt)
            aux = singles.tile([RB, N_RB * D + RB], f32)
            idx_all = singles.tile([RB, N_RB * K], u32)
            nc.sync.dma_start(out=mat, in_=p_mat[:, :])
            nc.sync.dma_start(out=aux, in_=p_aux[:, :])
            lhsT = mat[:, :ROWS_PER_CORE]
            rhs = mat[:, ROWS_PER_CORE:]
            qrows = aux[:, : N_RB * D]
            diag = aux[:, N_RB * D :]
            # Pre-touch aux on DVE: absorbs the aux-DMA dependency into DVE
            # program order so the per-rb diag add needs no extra sync wait.
            pre = small.tile([RB, 8], f32, tag="pre")
            nc.vector.tensor_copy(pre, aux[:, :8])

            for _rep in range(loop_reps):
                for rb in range(N_RB):
                    m_tile = mpool.tile([RB, N], f32, tag="m")
                    for c in range(N_CHUNK):
                        ps = pp.tile([RB, CHUNK], f32, tag="ps")
                        nc.tensor.matmul(
                            out=ps,
                            lhsT=lhsT[:, rb * RB : (rb + 1) * RB],
                            rhs=rhs[:, c * CHUNK : (c + 1) * CHUNK],
                            start=True,
                            stop=True,
                        )
                        nc.scalar.activation(
                            out=m_tile[:, c * CHUNK : (c + 1) * CHUNK],
                            in_=ps,
                            func=mybir.ActivationFunctionType.Copy,
                        )
                    # mask self-distance on the block diagonal
                    nc.vector.tensor_add(
                        out=m_tile[:, rb * RB : (rb + 1) * RB],
                        in0=m_tile[:, rb * RB : (rb + 1) * RB],
                        in1=diag,
                    )
                    vals = small.tile([RB, K], f32, tag="vals")
                    nc.vector.max(out=vals, in_=m_tile)
                    nc.vector.max_index(
                        out=idx_all[:, rb * K : (rb + 1) * K],
                        in_max=vals,
                        in_values=m_tile,
                    )

            # Gather q rows for all neighbors (one offset per partition per
            # call; see module docstring).  Explicit semaphore: Tile's own
            # dependency tracking is unreliable for DynamicAP DMAs.
            tc.strict_bb_all_engine_barrier()
            gathered = singles.tile([RB, N_RB * K, D], f32)
            if for_sim:
                # TimelineSim cannot model DynamicAP completion; the gather
                # phase is accounted separately.
                nc.vector.memset(gathered, 0.0)
            else:
                for g in range(N_RB * K):
                    nc.gpsimd.indirect_dma_start(
                        out=gathered[:, g, :],
                        out_offset=None,
                        in_=p_q[:, :],
                        in_offset=bass.IndirectOffsetOnAxis(
                            ap=idx_all[:, g : g + 1], axis=0
                        ),
                    ).then_inc(gsem, 16)
                with tc.tile_critical():
                    nc.vector.wait_ge(gsem, 16 * N_RB * K)

            # neighbor sum, lap = sum/K - qrow, then L1 partial per partition
            nbr = small.tile([RB, N_RB, D], f32, tag="nbr")
            nc.vector.tensor_reduce(
                out=nbr,
                in_=gathered[:].rearrange("p (rb s) d -> p rb d s", rb=N_RB),
                axis=mybir.AxisListType.X,
                op=mybir.AluOpType.add,
            )
            lap = small.tile([RB, N_RB * D], f32, tag="lap")
            nc.vector.tensor_scalar(
                out=lap,
                in0=nbr[:].rearrange("p a b -> p (a b)"),
                scalar1=1.0 / K,
                scalar2=None,
                op0=mybir.AluOpType.mult,
            )
            nc.vector.tensor_sub(lap, lap, qrows)
            partial = small.tile([RB, 1], f32, tag="partial")
            nc.vector.tensor_reduce(
                out=partial,
                in_=lap,
                axis=mybir.AxisListType.X,
                op=mybir.AluOpType.add,
                apply_absolute_value=True,
            )
            ones = singles.tile([RB, 1], f32)
            nc.vector.memset(ones, 1.0)
            ps_out = pp_out.tile([1, 1], f32, tag="ps_out")
            nc.tensor.matmul(out=ps_out, lhsT=partial, rhs=ones, start=True, stop=True)
            out_sb = small.tile([1, 1], f32, tag="out_sb")
            nc.vector.tensor_copy(out_sb, ps_out)
            nc.sync.dma_start(out=o_partial[:, :], in_=out_sb)
            nc.sync.dma_start(out=o_idx[:, :], in_=idx_all)

    nc.compile()
    return nc


def _trunc10(x):
    """Zero the low 13 mantissa bits: exactly representable in float32r."""
    return (np.asarray(x, np.float32).view(np.uint32) & np.uint32(0xFFFFE000)).view(
        np.float32
    )


def make_in_maps(point1: np.ndarray, point2: np.ndarray):
    in_maps = []
    for core in range(N_CORES):
        b = core // 2
        half = core % 2
        r0 = half * ROWS_PER_CORE
        rows = slice(r0, r0 + ROWS_PER_CORE)
        x = point1[b].astype(np.float32)  # (N, D)
        hi = _trunc10(x)
        lo = _trunc10(x - hi)
        sq = (x.astype(np.float64) ** 2).sum(axis=1).astype(np.float32)
        sqhi = _trunc10(sq)
        sqlo = _trunc10(sq - sqhi)
        rot = (np.arange(N) + r0) % N  # local column jj -> global column

        mat = np.empty((MM_K, ROWS_PER_CORE + N), np.float32)
        L, R = mat[:, :ROWS_PER_CORE], mat[:, ROWS_PER_CORE:]
        # 2<p_i,p_j> - sq_j - sq_i  with hi/lo splits (lo.lo dropped)
        L[0:3] = hi[rows].T
        R[0:3] = 2.0 * hi[rot].T
        L[3:6] = hi[rows].T
        R[3:6] = 2.0 * lo[rot].T
        L[6:9] = lo[rows].T
        R[6:9] = 2.0 * hi[rot].T
        L[9] = 1.0
        R[9] = -sqhi[rot]
        L[10] = 1.0
        R[10] = -sqlo[rot]
        L[11] = sqhi[rows]
        R[11] = -1.0
        L[12] = sqlo[rows]
        R[12] = -1.0

        q = (point1[b] - point2[b]).astype(np.float32)[rot]  # rotated (N, D)
        qr = (point1[b] - point2[b]).astype(np.float32)[rows]
        qrows = qr.reshape(N_RB, RB, D).transpose(1, 0, 2).reshape(RB, N_RB * D)

        diag = np.zeros((RB, RB), np.float32)
        np.fill_diagonal(diag, NEG_BIG)
        aux = np.concatenate([qrows, diag], axis=1)

        in_maps.append({"mat": mat, "q": q, "aux": np.ascontiguousarray(aux)})
    return in_maps


def _get_nc():
    if "nc" not in _CACHED:
        _CACHED["nc"] = build_nc()
    return _CACHED["nc"]


def run(point1, point2, trace=False):
    nc = _get_nc()
    in_maps = make_in_maps(np.asarray(point1), np.asarray(point2))
    res = run_bass_kernel_spmd(nc, in_maps, list(range(N_CORES)), trace=trace)
    total = sum(float(r["partial"][0, 0]) for r in res.results)
    out = np.float32(total / (B * N * D))
    return out, res


def kernel(point1: np.ndarray, point2: np.ndarray) -> np.ndarray:
    out, _ = run(point1, point2, trace=False)
    return np.asarray(out)


if __name__ == "__main__":
    p1 = np.random.default_rng(0).normal(size=(B, N, D)).astype(np.float32)
    p2 = np.random.default_rng(1).normal(size=(B, N, D)).astype(np.float32)
    print(kernel(p1, p2))
